# revision 18
# baseline (speedup 1.0000x reference)
"""Trainium2 Bass kernel for the HGT message-passing + IQN head network.

Self-contained: takes full (unsharded) inputs as produced by the problem's
setup_inputs(), shards 16 graphs per core across 8 NeuronCores, runs a
Bass/Tile kernel via run_bass_kernel_spmd, and reassembles the full output.

Hardcoded problem shapes:
  B=128 graphs, 64 piles/graph (8192 piles), 1 crane/graph, x feature 88,
  HID=512, HEADS=4 (d=128), T=8 taus, N_COS=64, ACT=64, DEG=16.

Sharding: graphs 16i..16i+16 -> core i. All weights replicated.

Key host-side preparation (pure input/weight reshaping, no data-dependent
network math):
  - a_rel (scaled by p_rel/sqrt(d)) and m_rel are folded into the k/v
    projection weights (constant folding, done in float64).
  - The p2p edge list is converted to per-destination count matrices in a
    graph-pair layout so segment-softmax becomes dense 128x128 attention
    with multiplicity weights.
  - c2p has exactly one incoming edge per pile => attention weight is
    exactly 1.0 in fp32, so it is a broadcast add of the crane message.
"""

import numpy as np
from contextlib import ExitStack

import concourse.bass as bass
import concourse.tile as tile
from concourse import bacc, mybir
from concourse import bass_utils

P = 128
N_CORES = 8
B = 128                 # graphs total
GPC = 16                # graphs per core
PPG = 64                # piles per graph
NPC = GPC * PPG         # piles per core = 1024
NCC = GPC               # cranes per core = 16
IN0 = 88
HID = 512
H = 4
D = 128
T = 8
NCOS = 64
ACTD = 64
DEG = 16
NPAIR = 8               # graph pairs per core

f32 = mybir.dt.float32
f32r = mybir.dt.float32r
AF = mybir.ActivationFunctionType
ALU = mybir.AluOpType
AX = mybir.AxisListType

SELU_L = 1.0507009873554805
SELU_A = 1.6732632423543772

_PROGRAM_CACHE = {}


# ---------------------------------------------------------------------------
# device program
# ---------------------------------------------------------------------------

def _emit_selu(nc, sp, out_ap, in_ap, tag):
    """out = selu(in). in_ap may be PSUM or SBUF. 128-part tiles."""
    shape = [in_ap.shape[0], in_ap.free_size()]
    m = sp.tile(shape, f32, tag=f"{tag}_m")
    e = sp.tile(shape, f32, tag=f"{tag}_e")
    r = sp.tile(shape, f32, tag=f"{tag}_r")
    # m = min(x, 0); e = selu_a*exp(m) - selu_a;  r = relu(selu_l * x)
    nc.vector.tensor_scalar(m[:], in_ap, 0.0, None, ALU.min)
    nc.scalar.activation(e[:], m[:], AF.Exp)
    nc.scalar.activation(r[:], in_ap, AF.Relu, scale=SELU_L)
    nc.vector.tensor_scalar(e[:], e[:], SELU_L * SELU_A, -SELU_L * SELU_A,
                            ALU.mult, ALU.add)
    nc.vector.tensor_tensor(out_ap, r[:], e[:], op=ALU.add)


def _conv_layer(nc, ctx, tc, sp, pp, lname, kc, xTp_chunk, xTc_chunk,
                load_w, skip_alpha, ident, C_t, M64_t, dbg):
    """Emit one HGT conv layer. Returns (hTp tile [128, 4*1024],
    hTc tile [128, 4*16]) in transposed layout (channel-chunk major).

    kc: number of 128-rows input-channel chunks (1 for 88-dim, 4 for 512).
    xTp_chunk(ic) -> AP [kdim, 1024] transposed pile input chunk.
    xTc_chunk(ic) -> AP [kdim, 16] transposed crane input chunk.
    w_aps: dict name -> list of per-chunk APs [kdim, 512] (proj weights) or
           [128, 512] x4 for 'a_p'/'a_c'.
    """

    # ---- transposed projections for piles (p2c first; slots reused for p2p)
    qT = sp.tile([P, H * NPC], f32r, tag="qT")
    ktpcT = sp.tile([P, H * NPC], f32r, tag="ktT")
    vtpc = sp.tile([P, NPC // P * HID], f32r, tag="vt")

    def proj_T(out_t, w):
        for oc in range(H):
            for nch in range(2):
                ps = pp.tile([P, 512], f32, tag="ps")
                for ic in range(kc):
                    nc.tensor.matmul(
                        out=ps[:],
                        lhsT=w[ic][:, oc * P:(oc + 1) * P],
                        rhs=xTp_chunk(ic)[:, nch * 512:(nch + 1) * 512],
                        start=(ic == 0), stop=(ic == kc - 1))
                nc.vector.tensor_copy(
                    out=out_t[:, oc * NPC + nch * 512:oc * NPC + nch * 512 + 512],
                    in_=ps[:])

    def proj_nat(out_t, w):
        for nch in range(NPC // P):
            ps = pp.tile([P, 512], f32, tag="ps")
            for ic in range(kc):
                nc.tensor.matmul(
                    out=ps[:],
                    lhsT=xTp_chunk(ic)[:, nch * P:(nch + 1) * P],
                    rhs=w[ic][:],
                    start=(ic == 0), stop=(ic == kc - 1))
            nc.vector.tensor_copy(
                out=out_t[:, nch * HID:(nch + 1) * HID], in_=ps[:])

    proj_T(qT, load_w("q_p"))
    proj_T(ktpcT, load_w("ktpc"))
    proj_nat(vtpc, load_w("vtpc"))

    # ---- crane projections (transposed). qTc is padded to 32 cols/head
    # (zeros) because matmul dst partition count must be >= 32.
    qTc = sp.tile([P, H * 32], f32r, tag="qTc")
    nc.vector.tensor_copy(qTc[:], nc.const_aps.tensor(0.0, (P, H * 32)))
    vtcpT = sp.tile([P, H * NCC], f32r, tag="vtcpT")
    for pad, out_t, w in ((32, qTc, load_w("q_c")), (NCC, vtcpT, load_w("vtcp"))):
        for oc in range(H):
            ps = pp.tile([P, NCC], f32, tag="ps")
            for ic in range(kc):
                nc.tensor.matmul(
                    out=ps[:], lhsT=w[ic][:, oc * P:(oc + 1) * P],
                    rhs=xTc_chunk(ic)[:],
                    start=(ic == 0), stop=(ic == kc - 1))
            nc.vector.tensor_copy(
                out=out_t[:, oc * pad:oc * pad + NCC], in_=ps[:])

    # ---- p2c attention (crane aggregation), masked dense rows h*32+g
    e_pc = sp.tile([P, NPC], f32, tag="e_pc")
    nc.gpsimd.memset(e_pc[:], 0.0)
    for nch in range(2):
        for h in range(H):
            ps = pp.tile([32, 512], f32, tag="ps")
            nc.tensor.matmul(
                out=ps[:],
                lhsT=qTc[:, h * 32:(h + 1) * 32],
                rhs=ktpcT[:, h * NPC + nch * 512:h * NPC + nch * 512 + 512],
                start=True, stop=True)
            nc.scalar.activation(
                e_pc[h * 32:h * 32 + GPC, nch * 512:(nch + 1) * 512],
                ps[:GPC, :], AF.Exp)
    we_pc = sp.tile([P, NPC], f32, tag="we_pc")
    nc.vector.tensor_tensor(we_pc[:], e_pc[:], M64_t[:], op=ALU.mult)
    s_pc = sp.tile([P, 1], f32, tag="s_pc")
    nc.vector.reduce_sum(s_pc[:], we_pc[:], axis=AX.X)
    # +1e-30 keeps the unused 32-stride filler rows (s=0) finite; it is
    # below half-ulp of any real row sum so those are bit-unchanged.
    nc.vector.tensor_scalar(s_pc[:], s_pc[:], 1e-30, None, ALU.add)
    r_pc = sp.tile([P, 1], f32, tag="r_pc")
    nc.vector.reciprocal(r_pc[:], s_pc[:])
    p_pc = sp.tile([P, NPC], f32r, tag="p_pc")
    nc.vector.tensor_scalar(p_pc[:], we_pc[:], r_pc[:, :1], None, ALU.mult)
    if dbg is not None and lname == "c1":
        nc.sync.dma_start(dbg["e_pc"][:, :], e_pc[:])
    # transpose p_pc -> pTpc [128 j, (nch, h*32+g)]
    pTpc = sp.tile([P, NPC], f32r, tag="pTpc")
    for nch in range(NPC // P):
        tps = pp.tile([P, P], f32r, tag="ps_s")
        nc.tensor.transpose(tps[:], p_pc[:, nch * P:(nch + 1) * P], ident[:])
        nc.vector.tensor_copy(pTpc[:, nch * P:(nch + 1) * P], tps[:])
    # msg: agg_cT [128, (h,16)]
    agg_cT = sp.tile([P, H * NCC], f32, tag="agg_cT")
    for h in range(H):
        ps = pp.tile([P, NCC], f32, tag="ps")
        for nch in range(NPC // P):
            nc.tensor.matmul(
                out=ps[:],
                lhsT=vtpc[:, nch * HID + h * P:nch * HID + (h + 1) * P],
                rhs=pTpc[:, nch * P + h * 32:nch * P + h * 32 + GPC],
                start=(nch == 0), stop=(nch == NPC // P - 1))
        nc.vector.tensor_copy(agg_cT[:, h * NCC:(h + 1) * NCC], ps[:])

    # ---- p2p projections reuse the p2c slots
    ktppT = sp.tile([P, H * NPC], f32r, tag="ktT")
    vtpp = sp.tile([P, NPC // P * HID], f32r, tag="vt")
    proj_T(ktppT, load_w("ktpp"))
    proj_nat(vtpp, load_w("vtpp"))

    # ---- p2p attention per graph pair + c2p broadcast add -> aggT piles
    aggT = sp.tile([P, H * NPC], f32, tag="aggT")
    for pr in range(NPAIR):
        A = pp.tile([P, 512], f32, tag="A")
        for h in range(H):
            nc.tensor.matmul(
                out=A[:, h * P:(h + 1) * P],
                lhsT=qT[:, h * NPC + pr * P:h * NPC + (pr + 1) * P],
                rhs=ktppT[:, h * NPC + pr * P:h * NPC + (pr + 1) * P],
                start=True, stop=True)
        e = sp.tile([P, 512], f32, tag="e")
        nc.scalar.activation(e[:], A[:], AF.Exp)
        if dbg is not None and pr == 0:
            nc.sync.dma_start(dbg["A0" if lname == "c1" else "A0c2"][:, :], e[:])
        we = sp.tile([P, 512], f32, tag="we")
        nc.vector.tensor_tensor(
            we[:].rearrange("p (h j) -> p h j", h=H),
            e[:].rearrange("p (h j) -> p h j", h=H),
            C_t[:, pr * P:(pr + 1) * P].unsqueeze(1).to_broadcast([P, H, P]),
            op=ALU.mult)
        s = sp.tile([P, H], f32, tag="s")
        nc.vector.reduce_sum(s[:], we[:].rearrange("p (h j) -> p h j", h=H),
                             axis=AX.X)
        r = sp.tile([P, H], f32, tag="r")
        nc.vector.reciprocal(r[:], s[:])
        pw = sp.tile([P, 512], f32r, tag="pw")
        nc.vector.tensor_tensor(
            pw[:].rearrange("p (h j) -> p h j", h=H),
            we[:].rearrange("p (h j) -> p h j", h=H),
            r[:].unsqueeze(2).to_broadcast([P, H, P]),
            op=ALU.mult)
        pT = sp.tile([P, 512], f32r, tag="pT")
        for h in range(H):
            tps = pp.tile([P, P], f32r, tag="ps_s")
            nc.tensor.transpose(tps[:], pw[:, h * P:(h + 1) * P], ident[:])
            nc.vector.tensor_copy(pT[:, h * P:(h + 1) * P], tps[:])
        for h in range(H):
            mps = pp.tile([P, P], f32, tag="ps_s")
            nc.tensor.matmul(
                out=mps[:],
                lhsT=vtpp[:, pr * HID + h * P:pr * HID + (h + 1) * P],
                rhs=pT[:, h * P:(h + 1) * P],
                start=True, stop=True)
            # agg = msg_pp + vt_cp[crane of each pile]  (c2p, att == 1.0)
            dst = aggT[:, h * NPC + pr * P:h * NPC + (pr + 1) * P]
            nc.vector.tensor_tensor(
                dst.rearrange("p (g j) -> p g j", g=2),
                mps[:].rearrange("p (g j) -> p g j", g=2),
                vtcpT[:, h * NCC + 2 * pr:h * NCC + 2 * pr + 2]
                .unsqueeze(2).to_broadcast([P, 2, PPG]),
                op=ALU.add)

    if dbg is not None and lname == "c2":
        nc.sync.dma_start(dbg["qT2"][:, :], qT[:].bitcast(f32))
        nc.sync.dma_start(dbg["ktppT2"][:, :], ktppT[:].bitcast(f32))
        nc.sync.dma_start(dbg["vtpp2"][:, :], vtpp[:].bitcast(f32))
        nc.sync.dma_start(dbg["aggT2"][:, :], aggT[:])
        nc.sync.dma_start(dbg["vtcpT2"][:, :], vtcpT[:].bitcast(f32))

    # ---- piles: gelu -> a-proj -> (skip) -> selu -> hTp
    gT = sp.tile([P, H * NPC], f32r, tag="qT")
    for c in range(H):
        nc.scalar.activation(gT[:, c * NPC:(c + 1) * NPC],
                             aggT[:, c * NPC:(c + 1) * NPC], AF.Gelu)
    wa_p = load_w("a_p")
    hTp = sp.tile([P, H * NPC], f32r, tag=f"hTp_{lname}")
    for oc in range(H):
        for nch in range(2):
            ps = pp.tile([P, 512], f32, tag="ps")
            for ic in range(H):
                nc.tensor.matmul(
                    out=ps[:],
                    lhsT=wa_p[ic][:, oc * P:(oc + 1) * P],
                    rhs=gT[:, ic * NPC + nch * 512:ic * NPC + nch * 512 + 512],
                    start=(ic == 0), stop=(ic == 3))
            dst = hTp[:, oc * NPC + nch * 512:oc * NPC + nch * 512 + 512]
            if skip_alpha is None:
                _emit_selu(nc, sp, dst, ps[:], "sl")
            else:
                a = float(skip_alpha[0])
                bl = sp.tile([P, 512], f32, tag="bl")
                nc.vector.tensor_scalar(bl[:], ps[:], a, None, ALU.mult)
                xin = xTp_chunk(oc)[:, nch * 512:nch * 512 + 512]
                xs = sp.tile([P, 512], f32, tag="xs")
                nc.vector.tensor_scalar(xs[:], xin, 1.0 - a, None, ALU.mult)
                nc.vector.tensor_tensor(bl[:], bl[:], xs[:], op=ALU.add)
                _emit_selu(nc, sp, dst, bl[:], "sl")

    # ---- cranes: gelu -> a-proj -> (skip) -> selu -> hTc
    gTc = sp.tile([P, H * NCC], f32r, tag="gTc")
    nc.scalar.activation(gTc[:], agg_cT[:], AF.Gelu)
    wa_c = load_w("a_c")
    hTc = sp.tile([P, H * NCC], f32r, tag=f"hTc_{lname}")
    for oc in range(H):
        ps = pp.tile([P, NCC], f32, tag="ps")
        for ic in range(H):
            nc.tensor.matmul(
                out=ps[:], lhsT=wa_c[ic][:, oc * P:(oc + 1) * P],
                rhs=gTc[:, ic * NCC:(ic + 1) * NCC],
                start=(ic == 0), stop=(ic == 3))
        dst = hTc[:, oc * NCC:(oc + 1) * NCC]
        if skip_alpha is None:
            _emit_selu(nc, sp, dst, ps[:], "slc")
        else:
            a = float(skip_alpha[1])
            bl = sp.tile([P, NCC], f32, tag="blc")
            nc.vector.tensor_scalar(bl[:], ps[:], a, None, ALU.mult)
            xin = xTc_chunk(oc)[:]
            xs = sp.tile([P, NCC], f32, tag="xsc")
            nc.vector.tensor_scalar(xs[:], xin, 1.0 - a, None, ALU.mult)
            nc.vector.tensor_tensor(bl[:], bl[:], xs[:], op=ALU.add)
            _emit_selu(nc, sp, dst, bl[:], "slc")
    return hTp, hTc


def build_program(skip_alpha_p, skip_alpha_c, debug=False):
    """Build and compile the SPMD Bass program (same on all 8 cores)."""
    nc = bacc.Bacc("TRN2", target_bir_lowering=False, debug=False,
                   enable_asserts=True, num_devices=N_CORES)

    def din(name, shape, dt=f32r):
        return nc.dram_tensor(name, shape, dt, kind="ExternalInput").ap()

    # per-core tensors
    xTp_d = din("xTp", [IN0, NPC])
    xTc_d = din("xTc", [IN0, NCC])
    C_d = din("Cw", [NPC, P], f32)
    taus_d = din("tausb", [64, GPC * T], f32)
    # shared constants
    M64_d = din("M64", [P, NPC], f32)
    ident_d = din("ident", [P, P])
    kvec_d = din("kvec", [64, 1], f32)
    # weights
    wd = {}
    for l, icdim in (("c1", IN0), ("c2", HID)):
        for wn in ("q_p", "ktpp", "ktpc", "vtpp", "vtpc", "q_c", "vtcp"):
            wd[f"{l}_{wn}"] = din(f"{l}_{wn}", [icdim, HID])
        for wn in ("a_p", "a_c"):
            wd[f"{l}_{wn}"] = din(f"{l}_{wn}", [HID, HID])
    wcos_d = din("wcos", [NCOS, HID])
    wff1_d = din("wff1", [HID, HID])
    wff2_d = din("wff2", [HID, ACTD])

    out_d = nc.dram_tensor("quant", [GPC * T, ACTD], f32,
                           kind="ExternalOutput").ap()
    dbg = None
    if debug:
        dbg = {}
        for name, shape in (("h1Tp", [P, H * NPC]), ("h2Tp", [P, H * NPC]),
                            ("h1Tc", [P, H * NCC]), ("h2Tc", [P, H * NCC]),
                            ("A0", [P, 512]), ("e_pc", [P, NPC]),
                            ("A0c2", [P, 512]), ("qT2", [P, H * NPC]),
                            ("ktppT2", [P, H * NPC]), ("vtpp2", [P, 8 * HID]),
                            ("aggT2", [P, H * NPC]), ("vtcpT2", [P, H * NCC]),
                            ("cosT", [64, GPC * T]), ("z1T", [P, H * GPC * T]),
                            ("hT", [P, H * GPC])):
            dbg[name] = nc.dram_tensor("dbg_" + name, shape, f32,
                                       kind="ExternalOutput").ap()

    with tile.TileContext(nc) as tc, ExitStack() as ctx:
        sp = ctx.enter_context(tc.tile_pool(name="sb", bufs=1))
        spw = ctx.enter_context(tc.tile_pool(name="sw", bufs=3))
        pp = ctx.enter_context(tc.tile_pool(name="pp", bufs=2, space="PSUM"))

        # --- load per-core inputs and constants
        xTp = sp.tile([IN0, NPC], f32r, tag="xTp")
        nc.sync.dma_start(xTp[:], xTp_d[:])
        xTc = sp.tile([IN0, NCC], f32r, tag="xTc")
        nc.sync.dma_start(xTc[:], xTc_d[:])
        C_t = sp.tile([P, NPAIR * P], f32, tag="C")
        for pr in range(NPAIR):
            nc.sync.dma_start(C_t[:, pr * P:(pr + 1) * P],
                              C_d[pr * P:(pr + 1) * P, :])
        M64_t = sp.tile([P, NPC], f32, tag="M64")
        nc.sync.dma_start(M64_t[:], M64_d[:])
        ident = sp.tile([P, P], f32r, tag="ident")
        nc.sync.dma_start(ident[:], ident_d[:])
        kvec = sp.tile([64, 1], f32, tag="kvec")
        nc.sync.dma_start(kvec[:], kvec_d[:])
        tausb = sp.tile([64, GPC * T], f32, tag="tausb")
        nc.sync.dma_start(tausb[:], taus_d[:])

        def load_w(name, icdim):
            kc = (icdim + P - 1) // P
            wt = spw.tile([min(icdim, P), kc * HID], f32r, tag="w")
            aps = []
            for ic in range(kc):
                lo = ic * P
                hi = min(icdim, lo + P)
                nc.sync.dma_start(wt[:hi - lo, ic * HID:ic * HID + HID],
                                  wd[name][lo:hi, :])
                aps.append(wt[:hi - lo, ic * HID:ic * HID + HID])
            return aps

        # --- conv1
        def lw1(k):
            return load_w(f"c1_{k}", HID if k in ("a_p", "a_c") else IN0)
        h1Tp, h1Tc = _conv_layer(
            nc, ctx, tc, sp, pp, "c1", 1,
            lambda ic: xTp[:], lambda ic: xTc[:],
            lw1, None, ident, C_t, M64_t, dbg)
        if debug:
            nc.sync.dma_start(dbg["h1Tp"][:, :], h1Tp[:].bitcast(f32))
            nc.sync.dma_start(dbg["h1Tc"][:, :], h1Tc[:].bitcast(f32))

        # --- conv2
        def lw2(k):
            return load_w(f"c2_{k}", HID)
        h2Tp, h2Tc = _conv_layer(
            nc, ctx, tc, sp, pp, "c2", H,
            lambda ic: h1Tp[:, ic * NPC:(ic + 1) * NPC],
            lambda ic: h1Tc[:, ic * NCC:(ic + 1) * NCC],
            lw2, (skip_alpha_p, skip_alpha_c), ident, C_t, M64_t, dbg)
        if debug:
            nc.sync.dma_start(dbg["h2Tp"][:, :], h2Tp[:].bitcast(f32))
            nc.sync.dma_start(dbg["h2Tc"][:, :], h2Tc[:].bitcast(f32))

        # --- global add pool + crane residual -> hT [128, (4,16)]
        hT = sp.tile([P, H * GPC], f32, tag="hT")
        for c in range(H):
            pl = sp.tile([P, GPC], f32, tag="pool")
            nc.vector.reduce_sum(
                pl[:],
                h2Tp[:, c * NPC:(c + 1) * NPC]
                .rearrange("p (g j) -> p g j", g=GPC),
                axis=AX.X)
            nc.vector.tensor_tensor(hT[:, c * GPC:(c + 1) * GPC], pl[:],
                                    h2Tc[:, c * NCC:(c + 1) * NCC], op=ALU.add)
        if debug:
            nc.sync.dma_start(dbg["hT"][:, :], hT[:])

        # --- IQN head (128 rows = (g,t) per core)
        NR = GPC * T  # 128 rows
        # cos features, transposed [64 k, 128 rows]
        tm = sp.tile([64, NR], f32, tag="tm")
        nc.vector.tensor_tensor(tm[:], kvec[:].to_broadcast([64, NR]),
                                tausb[:], op=ALU.mult)
        # t2 = k*tau/2 + 1/4; f = t2 - round(t2) in [-.5, .5] via the
        # +2^23 trick; sin(2*pi*f) == cos(pi*k*tau) exactly.
        nc.vector.tensor_scalar(tm[:], tm[:], 0.25, None, ALU.add)
        rt = sp.tile([64, NR], f32, tag="rt")
        nc.vector.tensor_scalar(rt[:], tm[:], float(2 ** 23), -float(2 ** 23),
                                ALU.add, ALU.add)
        nc.vector.tensor_tensor(tm[:], tm[:], rt[:], op=ALU.subtract)
        cosT = sp.tile([64, NR], f32r, tag="cosT")
        nc.scalar.activation(cosT[:], tm[:], AF.Sin, scale=float(2 * np.pi))
        if debug:
            nc.sync.dma_start(dbg["cosT"][:, :], cosT[:].bitcast(f32))
        wcos = spw.tile([64, HID], f32r, tag="w")
        nc.sync.dma_start(wcos[:], wcos_d[:])
        # z1T = selu(cosT-proj) * hT broadcast ; [128, (4,128)]
        z1T = sp.tile([P, H * NR], f32r, tag="z1T")
        for oc in range(H):
            ps = pp.tile([P, NR], f32, tag="ps")
            nc.tensor.matmul(out=ps[:], lhsT=wcos[:, oc * P:(oc + 1) * P],
                            rhs=cosT[:], start=True, stop=True)
            cx = sp.tile([P, NR], f32, tag="cx")
            _emit_selu(nc, sp, cx[:], ps[:], "slh")
            nc.vector.tensor_tensor(
                z1T[:, oc * NR:(oc + 1) * NR]
                .rearrange("p (g t) -> p g t", g=GPC),
                cx[:].rearrange("p (g t) -> p g t", g=GPC),
                hT[:, oc * GPC:(oc + 1) * GPC]
                .unsqueeze(2).to_broadcast([P, GPC, T]),
                op=ALU.mult)
        if debug:
            nc.sync.dma_start(dbg["z1T"][:, :], z1T[:].bitcast(f32))
        # z2T = selu(ff1^T z1T)
        wff1 = spw.tile([P, H * HID], f32r, tag="w")
        for ic in range(H):
            nc.sync.dma_start(wff1[:, ic * HID:(ic + 1) * HID],
                              wff1_d[ic * P:(ic + 1) * P, :])
        z2T = sp.tile([P, H * NR], f32r, tag="z2T")
        for oc in range(H):
            ps = pp.tile([P, NR], f32, tag="ps")
            for ic in range(H):
                nc.tensor.matmul(
                    out=ps[:],
                    lhsT=wff1[:, ic * HID + oc * P:ic * HID + (oc + 1) * P],
                    rhs=z1T[:, ic * NR:(ic + 1) * NR],
                    start=(ic == 0), stop=(ic == 3))
            _emit_selu(nc, sp, z2T[:, oc * NR:(oc + 1) * NR], ps[:], "slh")
        # out = z2 @ ff2, natural layout [128 rows, 64]
        wff2 = spw.tile([P, H * ACTD], f32r, tag="wff2")
        for ic in range(H):
            nc.sync.dma_start(wff2[:, ic * ACTD:(ic + 1) * ACTD],
                              wff2_d[ic * P:(ic + 1) * P, :])
        ps = pp.tile([P, ACTD], f32, tag="ps")
        for ic in range(H):
            nc.tensor.matmul(out=ps[:], lhsT=z2T[:, ic * NR:(ic + 1) * NR],
                            rhs=wff2[:, ic * ACTD:(ic + 1) * ACTD],
                            start=(ic == 0), stop=(ic == 3))
        onat = sp.tile([P, ACTD], f32, tag="onat")
        nc.vector.tensor_copy(onat[:], ps[:])
        nc.sync.dma_start(out_d[:, :], onat[:])

    nc.compile()
    return nc


# ---------------------------------------------------------------------------
# host side
# ---------------------------------------------------------------------------

def _fold_w(Wk, rel, mult):
    """fold per-head [d,d] rel matrices (x mult scalar) into [in,512] W."""
    Wk = np.asarray(Wk, np.float64)
    rel = np.asarray(rel, np.float64)
    icd = Wk.shape[0]
    W = Wk.reshape(icd, H, D)
    out = np.einsum("ihd,hdf->ihf", W, rel * np.asarray(mult, np.float64)[:, None, None])
    return np.ascontiguousarray(out.reshape(icd, HID), dtype=np.float32)


def _prep_weights(params):
    scale = 1.0 / np.sqrt(D)
    out = {}
    for l, cname in (("c1", "conv1"), ("c2", "conv2")):
        p = params[cname]
        nt, et = p["nt"], p["et"]
        for t, suf in (("pile", "p"), ("crane", "c")):
            for wn in ("k", "q", "v", "a"):
                assert np.max(np.abs(np.asarray(nt[t][wn]["b"]))) == 0.0, \
                    "nonzero bias unsupported"
        out[f"{l}_q_p"] = np.ascontiguousarray(
            np.asarray(nt["pile"]["q"]["W"], np.float32))
        out[f"{l}_q_c"] = np.ascontiguousarray(
            np.asarray(nt["crane"]["q"]["W"], np.float32))
        out[f"{l}_a_p"] = np.ascontiguousarray(
            np.asarray(nt["pile"]["a"]["W"], np.float32))
        out[f"{l}_a_c"] = np.ascontiguousarray(
            np.asarray(nt["crane"]["a"]["W"], np.float32))
        out[f"{l}_ktpp"] = _fold_w(nt["pile"]["k"]["W"], et["p2p"]["a_rel"],
                                   np.asarray(et["p2p"]["p_rel"]) * scale)
        out[f"{l}_ktpc"] = _fold_w(nt["pile"]["k"]["W"], et["p2c"]["a_rel"],
                                   np.asarray(et["p2c"]["p_rel"]) * scale)
        out[f"{l}_vtpp"] = _fold_w(nt["pile"]["v"]["W"], et["p2p"]["m_rel"],
                                   np.ones(H))
        out[f"{l}_vtpc"] = _fold_w(nt["pile"]["v"]["W"], et["p2c"]["m_rel"],
                                   np.ones(H))
        out[f"{l}_vtcp"] = _fold_w(nt["crane"]["v"]["W"], et["c2p"]["m_rel"],
                                   np.ones(H))
    # sigmoid(skip) blend factors
    sig = lambda x: 1.0 / (1.0 + np.exp(-np.float64(x)))
    alphas = (float(sig(params["conv2"]["nt"]["pile"]["skip"])),
              float(sig(params["conv2"]["nt"]["crane"]["skip"])))
    assert np.asarray(params["cos_emb"]["b"]).max() == 0.0
    assert np.abs(np.asarray(params["ff1"]["b"])).max() == 0.0
    assert np.abs(np.asarray(params["ff2"]["b"])).max() == 0.0
    out["wcos"] = np.ascontiguousarray(
        np.asarray(params["cos_emb"]["W"], np.float32))
    out["wff1"] = np.ascontiguousarray(np.asarray(params["ff1"]["W"], np.float32))
    out["wff2"] = np.ascontiguousarray(np.asarray(params["ff2"]["W"], np.float32))
    return out, alphas


def _prep_shards(x_crane, x_pile, ei_pc, ei_cp, ei_pp, taus):
    x_crane = np.asarray(x_crane, np.float32)
    x_pile = np.asarray(x_pile, np.float32)
    taus = np.asarray(taus, np.float32)
    ei_pc = np.asarray(ei_pc)
    ei_cp = np.asarray(ei_cp)
    ei_pp = np.asarray(ei_pp)
    NPT = B * PPG
    pid = np.arange(NPT, dtype=ei_pc.dtype)
    assert np.array_equal(ei_pc[0], pid) and np.array_equal(ei_pc[1], pid // PPG), \
        "unexpected p2c edge structure"
    assert np.array_equal(ei_cp[0], pid // PPG) and np.array_equal(ei_cp[1], pid), \
        "unexpected c2p edge structure"
    src, dst = ei_pp[0].astype(np.int64), ei_pp[1].astype(np.int64)
    assert np.array_equal(src // PPG, dst // PPG), \
        "p2p edges must stay within a graph"
    # count matrix [8192, 64]
    Cfull = np.bincount(dst * PPG + (src % PPG),
                        minlength=NPT * PPG).reshape(NPT, PPG)
    # graph-pair layout [8192, 128]
    Cp = np.zeros((NPT, P), np.float32)
    gpar = (np.arange(NPT) // PPG) % 2
    Cp[gpar == 0, :PPG] = Cfull[gpar == 0]
    Cp[gpar == 1, PPG:] = Cfull[gpar == 1]

    shards = []
    for c in range(N_CORES):
        s = {}
        s["xTp"] = np.ascontiguousarray(
            x_pile[c * NPC:(c + 1) * NPC].T)
        s["xTc"] = np.ascontiguousarray(
            x_crane[c * GPC:(c + 1) * GPC].T)
        s["Cw"] = np.ascontiguousarray(Cp[c * NPC:(c + 1) * NPC])
        tr = taus[c * GPC:(c + 1) * GPC, :, 0].reshape(1, GPC * T)
        s["tausb"] = np.ascontiguousarray(np.repeat(tr, 64, axis=0))
        shards.append(s)
    return shards, taus


def _prep_consts():
    M64 = np.zeros((P, NPC), np.float32)
    for h in range(H):
        for g in range(GPC):
            M64[h * 32 + g, g * PPG:(g + 1) * PPG] = 1.0
    ident = np.eye(P, dtype=np.float32)
    kvec = (np.arange(1, NCOS + 1, dtype=np.float32) / 2.0).reshape(NCOS, 1)
    return {"M64": M64, "ident": ident, "kvec": kvec}


def _ensure_ntff_hook():
    """Register the NTFF profile hook that this image's antenv lacks."""
    try:
        from antenv import axon_hooks  # noqa: F401
        return True
    except ImportError:
        pass
    try:
        import sys, types, os
        import antenv
        from trn_agent_boot.trn_boot import _ntff_profile_via_ctypes
        so = "/opt/axon/libaxon_pjrt.so"
        if not os.path.exists(so):
            return False
        hook = _ntff_profile_via_ctypes(so)
        if hook is None:
            return False
        mod = types.ModuleType("antenv.axon_hooks")
        state = {"hook": hook}
        mod.set_axon_ntff_profile_hook = lambda h: state.__setitem__("hook", h)
        mod.get_axon_ntff_profile_hook = lambda: state["hook"]
        sys.modules["antenv.axon_hooks"] = mod
        antenv.axon_hooks = mod
        # artifact upload needs bucket creds this sandbox lacks
        bass_utils.upload_artifacts = lambda tmpdir: tmpdir
        return True
    except Exception:
        return False


def _patch_sim_gelu():
    """CoreSim has no Gelu; add an erf-based one (matches HW to ~4e-7)."""
    from concourse import bass_interp as bi
    import concourse.mybir as mb
    import scipy.special as sps
    cls = bi.InstructionExecutor
    if getattr(cls, "_gelu_patched", False):
        return
    orig = cls.visit_InstActivation

    def visit(self, instruction, *, reg_snapshot=None):
        if instruction.func != mb.ActivationFunctionType.Gelu:
            return orig(self, instruction, reg_snapshot=reg_snapshot)
        input_ap, bias, scale = instruction.ins[0], instruction.ins[1], instruction.ins[2]
        output_ap = instruction.outs[0]
        iv = self.view_ap(input_ap, bi.Direction.READ, instruction,
                          reg_snapshot=reg_snapshot).astype(np.float32)
        iv = iv.reshape(iv.shape[0], -1)
        if isinstance(bias, mb.ImmediateValue):
            bv = bias.value
        else:
            bv = self.view_ap(bias, bi.Direction.READ, instruction,
                              reg_snapshot=reg_snapshot).astype(np.float32)
            bv = bv.reshape(bv.shape[0], -1)
        sv = scale.value if isinstance(scale, mb.ImmediateValue) else None
        x = iv * sv + bv
        acted = (x * 0.5 * (1.0 + sps.erf(x / np.sqrt(2.0)))).astype(np.float32)
        ov = self.view_ap(output_ap, bi.Direction.WRITE, instruction,
                          reg_snapshot=reg_snapshot)
        ov[:] = acted.reshape(ov.shape)

    cls.visit_InstActivation = visit
    cls._gelu_patched = True


def kernel(x_crane, x_pile, ei_pc, ei_cp, ei_pp, taus, params,
           _debug=False, _sim=False, _trace=False):
    w, alphas = _prep_weights(params)
    shards, taus_np = _prep_shards(x_crane, x_pile, ei_pc, ei_cp, ei_pp, taus)
    consts = _prep_consts()

    key = (alphas, _debug)
    if key not in _PROGRAM_CACHE:
        _PROGRAM_CACHE[key] = build_program(alphas[0], alphas[1], debug=_debug)
    nc = _PROGRAM_CACHE[key]

    in_maps = []
    for c in range(N_CORES):
        m = {}
        m.update(shards[c])
        m.update(consts)
        m.update(w)
        in_maps.append(m)

    if _sim:
        from concourse.bass_interp import CoreSim
        _patch_sim_gelu()
        results = []
        for c in range(N_CORES if _sim is True else 1):
            sim = CoreSim(nc, trace=False)
            for name, val in in_maps[c].items():
                sim.tensor(name)[:] = val
            sim.simulate(check_with_hw=False)
            outs = {"quant": np.array(sim.tensor("quant"))}
            if _debug:
                for name in ("h1Tp", "h2Tp", "h1Tc", "h2Tc", "A0", "e_pc",
                             "A0c2", "qT2", "ktppT2", "vtpp2", "aggT2",
                             "vtcpT2", "cosT", "z1T", "hT"):
                    outs["dbg_" + name] = np.array(sim.tensor("dbg_" + name))
            results.append(outs)
        res = None
    else:
        if _trace and not _ensure_ntff_hook():
            _trace = False
        res = bass_utils.run_bass_kernel_spmd(
            nc, in_maps, core_ids=list(range(N_CORES)), trace=_trace)
        results = res.results

    quant = np.concatenate(
        [results[c]["quant"].reshape(GPC, T, ACTD) for c in range(len(results))],
        axis=0)
    if _debug or (_sim and len(results) < N_CORES):
        kernel._last_results = results
    kernel._last_res = res
    return quant, taus_np
